# revision 56
# baseline (speedup 1.0000x reference)
"""Soft-MoE discrete-action transition network — Trainium2 Bass kernel.

Problem shapes (hardcoded):
  obs [B=64, M=256, D=256] f32, action [B=64] i64,
  phi [D, E=4, S=64] f32, w1 [E, D, H=512] f32, b1 [E, H] f32 (zeros),
  w2 [E, H, A*D=4608] f32, b2 [E, A*D] f32 (zeros).  Output [B, M, D] f32.

Strategy (v2):
  * Host gathers the action-selected slice of w2 per batch
    (w2sel[b] = w2[:, :, a_b*D:(a_b+1)*D]) — the one-hot contraction at the
    end of the reference selects exactly one D-wide block per batch.
  * Data-parallel over batch: 8 batch elements per NeuronCore. Batches with
    EQUAL actions are grouped so one w2sel tile serves the whole group: the
    host solves for a static per-core group pattern (e.g. (4,2,1,1) = one
    quad + one pair + two singles = 4 w2 tiles/core instead of 8), halving
    the dominant DMA traffic. The pattern is compiled into the program; the
    host picks the densest feasible pattern for the actual action multiset.
  * obs+obsT ride one DMA per batch; output is stored fp16 (converted back
    to f32 on host). All layout rearrangement happens on the host so every
    device DMA is a contiguous [128, N] copy.
  * Matmul operands are fp16; PSUM accumulation and softmax plumbing fp32.
  * Per batch, on device (P=128 partition chunks):
      logits  [m,es] = obsT.T @ phi      (lhsT=obsT[d,m], rhs=phi[d,es])
      logitsT [es,m] = phi.T  @ obsT     (same operands)
      exp both (ScalarE; accum_out yields both softmax denominators free)
      slotsT  [d,es] = obs.T @ exp_l     (unnormalized dispatch)
      pre_h   [h,es] = w1_e.T @ slotsT   per expert; ReLU (dispatch softmax
                        normalizer folded past ReLU — valid since b1 == 0;
                        nonzero b1 falls back to an exact host computation)
      yT      [d,es] = w2sel_e.T @ h_e   per (expert, d-chunk): free dim 64
                        instead of 256 — half the PE rows of the y-major
                        form; then PE-transpose (4x 128x128 fp16) to y[es,d]
                        and scale rows by 1/colsum (dispatch) on the
                        PSUM->SBUF copy.
      out     [m,d]  = exp_lT.T @ y; scale rows by 1/rowsum (combine)
  * Schedule (tuned against the TimelineSim cost model, 52.7us -> 43.6us):
    - 3-stage software pipeline (stage1 logits/dispatch, stage2 expert MLP,
      stage3 combine/store) with stage1 running 3 batches ahead; each stage
      owns its PSUM tags so depth costs no extra PSUM (8 banks exactly).
    - A tiny PE warm-up (5 identity transposes off the const identity tile)
      starts the cost model's p-state ramp clock during the first DMAs, so
      every real matmul runs at the full 2.4 GHz rate.
    - w2 group loads are scheduled between the oo loads (not before batch
      2, not after their consumer's emission); w1 loads after oo1.
    - PSUM->SBUF traffic is balanced across DVE and ACT (ReLU half on each;
      exps on ACT; slots/yT/y/out copies on DVE); mid-stream output stores
      issue from the idle GPSIMD queue (SWDGE) to keep HWDGE free; the last
      batch stores per-half from SP and splits its copies across engines.
"""

import os
import sys
import time

import numpy as np

for _p in ("/opt/trn_rl_repo",):
    if os.path.isdir(_p) and _p not in sys.path:
        sys.path.append(_p)

import concourse.bass as bass
import concourse.mybir as mybir
import concourse.tile as tile
from concourse import bacc
from concourse.bass import ds, ts
from concourse.masks import make_identity

B, M, D, A = 64, 256, 256, 18
E, S, H = 4, 64, 512
ES = E * S
N_CORES = 8
BPC = B // N_CORES  # batches per core
P = 128
F32 = mybir.dt.float32
F16 = mybir.dt.float16

AF = mybir.ActivationFunctionType

MM_DT = getattr(mybir.dt, os.environ.get("MOE_MM_DT", "float16"))
Y_DT = getattr(mybir.dt, os.environ.get("MOE_Y_DT", "float16"))

# Candidate per-core group patterns, densest first. Each tuple sums to BPC;
# all 8 cores share the pattern, so a pattern needs 8 equal-action groups of
# every size >= 2 to exist in the global action multiset.
PATTERNS = [
    (4, 2, 1, 1), (3, 3, 1, 1), (3, 2, 2, 1), (2, 2, 2, 2), (5, 2, 1),
    (4, 2, 2), (4, 3, 1), (3, 3, 2), (2, 2, 2, 1, 1), (3, 2, 1, 1, 1),
    (4, 1, 1, 1, 1), (2, 2, 1, 1, 1, 1), (3, 1, 1, 1, 1, 1),
    (2, 1, 1, 1, 1, 1, 1), (1, 1, 1, 1, 1, 1, 1, 1),
]


def solve_groups(action):
    """Pick the densest feasible pattern; return (pattern, order) where
    order[c*BPC + i] = original batch index placed at core c, slot i.
    Slots are laid out group-by-group following the pattern."""
    from collections import Counter

    by_action = {}
    for idx, a in enumerate(np.asarray(action).tolist()):
        by_action.setdefault(a, []).append(idx)

    for pat in PATTERNS:
        cnt = Counter({a: len(v) for a, v in by_action.items()})
        need = sorted((s for s in pat for _ in range(N_CORES)), reverse=True)
        picks = []  # action per group, aligned with `need`
        ok = True
        for s in need:
            if s == 1:
                picks.append(None)  # singles take leftovers later
                continue
            a = max(cnt, key=lambda k: cnt[k])
            if cnt[a] < s:
                ok = False
                break
            cnt[a] -= s
            if cnt[a] == 0:
                del cnt[a]
            picks.append(a)
        if not ok:
            continue
        # materialize: pools of remaining batch indices per action
        pools = {a: list(v) for a, v in by_action.items()}
        groups_by_size = {}
        for s, a in zip(need, picks):
            if s == 1:
                continue
            groups_by_size.setdefault(s, []).append(pools[a][:s])
            pools[a] = pools[a][s:]
        leftovers = [i for v in pools.values() for i in v]
        li = 0
        order = []
        taken = {s: 0 for s in groups_by_size}
        for c in range(N_CORES):
            for s in pat:
                if s == 1:
                    order.append(leftovers[li])
                    li += 1
                else:
                    g = groups_by_size[s][taken[s]]
                    taken[s] += 1
                    order.extend(g)
        assert li == len(leftovers) and len(order) == B
        return pat, np.asarray(order)
    raise AssertionError("(1,)*BPC is always feasible")


def build_nc(pattern, mm_dt=F32, y_dt=None, has_b2=False, *, o_dt=F16,
             io_bufs=4, mid_bufs=3, w1_late=True,
             relu_eng=("vector", "scalar"), slots_eng="vector",
             ytc_eng="vector", ysc_eng="vector", out_eng=("vector", "vector"),
             last_out_eng=("vector", "vector"), store_eng="gpsimd",
             lookahead=3, warmup=5, w1_at=1, w2_load_at=None,
             split_first=False, last_par=True, last_ymajor=False, s3_lag=1,
             s1_order="interleaved", sl_share=False, ou_split=False,
             ou_order="interleave", yt_single=False, ytr_share=False,
             w1_split=False, fine_last=0, endgame=3, conv=(), yt2_ph=False,
             lp_lo=2, lp_hi=BPC):
    """Build the per-core Bass program (one NeuronCore, BPC batches).

    pattern: per-core group sizes, e.g. (4,2,1,1). Batch ib belongs to group
    g per the cumulative pattern; one w2 tile is loaded per group.
    """
    if y_dt is None:
        y_dt = mm_dt
    G = len(pattern)
    # batch index -> group index, and group start flags
    b2g = []
    for g, s in enumerate(pattern):
        b2g += [g] * s
    assert len(b2g) == BPC
    if w2_load_at is None:
        # Each group's load rides a stage1 near (but not before emission of)
        # its first consumer: at least batch 2 (so the early oo loads are not
        # starved), at most first_batch + lookahead (so the load is EMITTED
        # before stage2(first_batch) consumes the tile).
        w2_load_at = tuple(
            min(max(b2g.index(g), 2), b2g.index(g) + lookahead, BPC - 1)
            for g in range(G)
        )
    assert len(w2_load_at) == G
    for g in range(G):
        assert w2_load_at[g] <= b2g.index(g) + lookahead, (
            "w2 load emitted after its consumer"
        )

    nc = bacc.Bacc("TRN2", target_bir_lowering=False, debug=False)

    oo_d = nc.dram_tensor(
        "oo", [BPC, P, 4 * D], mm_dt, kind="ExternalInput"
    ).ap()
    phi_d = nc.dram_tensor("phi", [P, 2 * ES], mm_dt, kind="ExternalInput").ap()
    w1_d = nc.dram_tensor("w1", [P, 2 * E * H], mm_dt, kind="ExternalInput").ap()
    w2_d = nc.dram_tensor(
        "w2grp", [G, P, E * 4 * D], y_dt, kind="ExternalInput"
    ).ap()
    if has_b2:
        b2_d = nc.dram_tensor(
            "b2grp", [G, 1, E * D], y_dt, kind="ExternalInput"
        ).ap()
    out_d = nc.dram_tensor("out", [BPC, P, 2 * D], o_dt, kind="ExternalOutput").ap()

    def eng(name):
        return {"vector": nc.vector, "scalar": nc.scalar,
                "gpsimd": nc.gpsimd, "sync": nc.sync}[name]

    def scaled_copy(engname, out, in0, scale):
        if engname == "scalar":
            nc.scalar.activation(out, in0, AF.Copy, scale=scale)
        else:
            eng(engname).tensor_scalar_mul(out, in0=in0, scalar1=scale)

    def relu_op(engname, out, in0):
        if engname == "scalar":
            nc.scalar.activation(out, in0, AF.Relu)
        else:
            eng(engname).tensor_scalar_max(out, in0, 0.0)

    def split2(pairs, scale=None):
        """pairs: [(out0, in0), (out1, in1)] — half on DVE, half on ACT,
        in parallel, to halve the chain latency of a drain-batch copy."""
        (o0, i0), (o1, i1) = pairs
        if scale is None:
            nc.vector.tensor_copy(o0, i0)
            nc.scalar.activation(o1, i1, AF.Copy)
        else:
            nc.vector.tensor_scalar_mul(o0, in0=i0, scalar1=scale)
            nc.scalar.activation(o1, i1, AF.Copy, scale=scale)

    def split2_relu(pairs):
        (o0, i0), (o1, i1) = pairs
        nc.vector.tensor_scalar_max(o0, i0, 0.0)
        nc.scalar.activation(o1, i1, AF.Relu)

    with tile.TileContext(nc) as tc:
        with (
            tc.tile_pool(name="const", bufs=1) as const,
            tc.tile_pool(name="io", bufs=io_bufs) as io,
            tc.tile_pool(name="mid", bufs=mid_bufs) as mid,
            tc.tile_pool(name="psum", bufs=1, space="PSUM") as psp,
        ):
            phi_sb = const.tile([P, 2, ES], mm_dt)
            if split_first:
                # split the phi load per d-chunk so the first logits matmul
                # (needing only dc0 of phi+obsT) starts ~700ns earlier
                phi_v = phi_d.rearrange("p (c s) -> p c s", c=2)
                for dc in range(2):
                    nc.sync.dma_start(out=phi_sb[:, dc, :], in_=phi_v[:, dc, :])
            else:
                nc.sync.dma_start(out=phi_sb, in_=phi_d)
            ident = const.tile([P, P], y_dt)
            make_identity(nc, ident)
            if warmup:
                # PE p-state warm-up: the cost model runs matmuls at half
                # speed until the engine has been continuously busy for 3us.
                # A stream of identity transposes into scratch PSUM (same tag
                # as the y transposes, so no extra PSUM) ramps the PE while
                # the first DMAs are in flight.
                warm_ps = psp.tile(
                    [P, 4, D], y_dt, tag="yT" if ytr_share else "ytr"
                )
                for _ in range(warmup):
                    nc.tensor.transpose(warm_ps[:, 0, 0:P], ident, ident)
            w1_sb = const.tile([P, 2, E, H], mm_dt)
            if not w1_late:
                nc.sync.dma_start(out=w1_sb, in_=w1_d)
            w2_tiles = [
                const.tile([P, E, 4, D], y_dt, name=f"w2t{g}") for g in range(G)
            ]
            if has_b2:
                b2_tiles = [
                    const.tile([P, 2, D], mm_dt, name=f"b2t{g}") for g in range(G)
                ]

            def stage1(ib):
                # obs and obsT ride one DMA; host stores them adjacently
                oo_sb = io.tile([P, 4, D], mm_dt, tag="oo")
                oo_v = oo_d[ib].rearrange("p (c d) -> p c d", c=4)
                if ib in conv:
                    # early batches: the serial DMA engine is the pacer while
                    # PE/DVE idle, so load only obs (half the bytes) and
                    # build obsT on-device with PE transposes
                    nc.sync.dma_start(out=oo_sb[:, 0:2, :], in_=oo_v[:, 0:2, :])
                    obs_sb = oo_sb[:, 0:2, :]
                    obsT_ps = psp.tile([P, 4, D], y_dt, tag="ytr")
                    for mc in range(2):
                        for dc in range(2):
                            nc.tensor.transpose(
                                obsT_ps[:, dc, ts(mc, P)],
                                obs_sb[:, mc, ts(dc, P)],
                                ident,
                            )
                    obsT_sb = oo_sb[:, 2:4, :]
                    for dc in range(2):
                        nc.vector.tensor_copy(
                            obsT_sb[:, dc, :], obsT_ps[:, dc, :]
                        )
                elif ib == 0 and split_first:
                    # obsT half first (gates the first logits matmuls),
                    # obs half after (only needed by the slots matmuls)
                    nc.sync.dma_start(out=oo_sb[:, 2:4, :], in_=oo_v[:, 2:4, :])
                    nc.sync.dma_start(out=oo_sb[:, 0:2, :], in_=oo_v[:, 0:2, :])
                    obs_sb = oo_sb[:, 0:2, :]
                    obsT_sb = oo_sb[:, 2:4, :]
                else:
                    nc.sync.dma_start(out=oo_sb, in_=oo_v)
                    obs_sb = oo_sb[:, 0:2, :]
                    obsT_sb = oo_sb[:, 2:4, :]
                if w1_late and ib in (
                    (w1_at, w1_at + 1) if w1_split else (w1_at,)
                ):
                    # logits only need phi+obsT; deferring the w1 const load
                    # past the first oo loads lets PE start earlier. With
                    # w1_split the h-halves arrive separately so the first
                    # preh chunks can start off the first half.
                    w1_v = w1_d.rearrange("p (c e h) -> p c e h", c=2, e=E)
                    if not w1_split:
                        nc.sync.dma_start(out=w1_sb, in_=w1_d)
                    else:
                        hh = ib - w1_at
                        nc.sync.dma_start(
                            out=w1_sb[:, :, :, ds(hh * H // 2, H // 2)],
                            in_=w1_v[:, :, :, ds(hh * H // 2, H // 2)],
                        )
                g = b2g[ib]
                # w2 tiles are independent const tiles, so their loads can be
                # scheduled freely (w2_load_at[g] = stage1 index issuing the
                # load) — keeps the late groups' loads off the pipeline-drain
                # critical path without starving the early oo loads.
                for g_ in [gi for gi, lb in enumerate(w2_load_at) if lb == ib]:
                    nc.sync.dma_start(out=w2_tiles[g_], in_=w2_d[g_].rearrange(
                        "p (e k) -> p e k", e=E))
                    if has_b2:
                        # broadcast b2grp[g][e] across the 64 slot partitions
                        # of each expert (pg = e % 2): 0-stride partition DMAs
                        for pg in range(2):
                            srcap = bass.AP(
                                tensor=b2_d.tensor,
                                offset=g_ * E * D + pg * D,
                                ap=[[0, S], [2 * D, 2], [1, D]],
                            )
                            nc.sync.dma_start(
                                out=b2_tiles[g_][pg * S : (pg + 1) * S, :, :],
                                in_=srcap,
                            )

                # logits [m, es] and logitsT [es, m], chunk-interleaved so
                # the first exp (and the slot matmuls) start earlier
                lg_ps = psp.tile([P, 2, ES], F32, tag="lg")
                lgT_ps = psp.tile([P, 2, M], F32, tag="lgT")
                exp_l = mid.tile([P, 2, ES], mm_dt, tag="expl")
                exp_lT = mid.tile([P, 2, M], mm_dt, tag="explT")
                sums = mid.tile([P, 4], F32, tag="sums")
                def lg_mm(c):
                    for dc in range(2):
                        nc.tensor.matmul(
                            lg_ps[:, c, :],
                            obsT_sb[:, dc, ts(c, P)],
                            phi_sb[:, dc, :],
                            start=(dc == 0),
                            stop=(dc == 1),
                        )

                def lgT_mm(c):
                    for dc in range(2):
                        nc.tensor.matmul(
                            lgT_ps[:, c, :],
                            phi_sb[:, dc, ts(c, P)],
                            obsT_sb[:, dc, :],
                            start=(dc == 0),
                            stop=(dc == 1),
                        )

                def exp_op(c):
                    nc.scalar.activation(
                        exp_l[:, c, :], lg_ps[:, c, :], AF.Exp,
                        accum_out=sums[:, c : c + 1],
                    )

                def expT_op(c):
                    nc.scalar.activation(
                        exp_lT[:, c, :], lgT_ps[:, c, :], AF.Exp,
                        accum_out=sums[:, 2 + c : 3 + c],
                    )

                if s1_order == "interleaved" or (
                    s1_order == "mm_first01" and ib > 1
                ):
                    # first exp starts earliest, but the c1 logits matmuls
                    # carry a PSUM-bank WAR on the c0 exps
                    for c in range(2):
                        lg_mm(c)
                        exp_op(c)
                        lgT_mm(c)
                        expT_op(c)
                elif s1_order == "lg_first":
                    # exp_l (gating the slots matmuls) first; exp_lT is not
                    # consumed until stage3, so its chain can trail
                    lg_mm(0)
                    exp_op(0)
                    lg_mm(1)
                    exp_op(1)
                    lgT_mm(0)
                    expT_op(0)
                    lgT_mm(1)
                    expT_op(1)
                else:
                    # all four matmul groups close their PSUM banks before
                    # any exp reads them: no WAR stalls on the PE
                    lg_mm(0)
                    lg_mm(1)
                    lgT_mm(0)
                    exp_op(0)
                    lgT_mm(1)
                    exp_op(1)
                    expT_op(0)
                    expT_op(1)

                # one reciprocal for both softmax denominators:
                # cols 0-1 = combine (per m-chunk), cols 2-3 = dispatch
                recips = mid.tile([P, 4], F32, tag="recips")
                nc.vector.reciprocal(recips, sums)
                recip_c = recips[:, 0:2]
                recip_d = recips[:, 2:4]

                # slotsT [d, es] = obs.T @ exp_l (unnormalized dispatch)
                sl_ps = psp.tile([P, 2, ES], F32, tag="lg" if sl_share else "sl")
                for dc in range(2):
                    for mc in range(2):
                        nc.tensor.matmul(
                            sl_ps[:, dc, :],
                            obs_sb[:, mc, ts(dc, P)],
                            exp_l[:, mc, :],
                            start=(mc == 0),
                            stop=(mc == 1),
                        )
                slots_sb = mid.tile([P, 2, ES], mm_dt, tag="slots")
                for eh in range(2):
                    if fine_last and ib >= BPC - fine_last:
                        # drain batches: halve the copy latency by running
                        # the dc halves on DVE and ACT in parallel
                        split2([
                            (slots_sb[:, 0, ts(eh, 2 * S)],
                             sl_ps[:, 0, ts(eh, 2 * S)]),
                            (slots_sb[:, 1, ts(eh, 2 * S)],
                             sl_ps[:, 1, ts(eh, 2 * S)]),
                        ])
                        continue
                    se = ("scalar" if eh and last_par
                          and BPC - lp_lo <= ib < lp_hi else slots_eng)
                    if se == "scalar":
                        nc.scalar.activation(
                            slots_sb[:, :, ts(eh, 2 * S)],
                            sl_ps[:, :, ts(eh, 2 * S)], AF.Copy,
                        )
                    else:
                        eng(se).tensor_copy(
                            slots_sb[:, :, ts(eh, 2 * S)],
                            sl_ps[:, :, ts(eh, 2 * S)],
                        )

                return (slots_sb, exp_lT, recip_c, recip_d, g)

            def stage2(ib, ctx):
                slots_sb, exp_lT, recip_c, recip_d, g = ctx
                w2_sb = w2_tiles[g]
                # pre_h [h, (e,s)]: h laid out [p, eh, hc, 2S] so each es-half
                # (2 experts) is an independent pipeline — its yT matmuls
                # start after its own ReLU, not after all experts' pre_h.
                h_sb = mid.tile([P, 2, 4, 2 * S], y_dt, tag="h")
                for eh in range(2):
                    ph_ps = psp.tile([P, 4, 2 * S], F32, tag="ph", bufs=2)
                    for hc in range(4):
                        for e2 in range(2):
                            e = 2 * eh + e2
                            for dc in range(2):
                                nc.tensor.matmul(
                                    ph_ps[:, hc, ds(e2 * S, S)],
                                    w1_sb[:, dc, e, ts(hc, P)],
                                    slots_sb[:, dc, ds(e * S, S)],
                                    start=(dc == 0),
                                    stop=(dc == 1),
                                )
                    if fine_last and ib >= BPC - fine_last:
                        split2_relu([
                            (h_sb[:, eh, 0:2], ph_ps[:, 0:2]),
                            (h_sb[:, eh, 2:4], ph_ps[:, 2:4]),
                        ])
                    else:
                        relu_op(relu_eng[eh], h_sb[:, eh], ph_ps)

                def h_slice(hc, e):
                    return h_sb[:, e // 2, hc, ds((e % 2) * S, S)]

                yT_ps = psp.tile([P, 2, ES], F32, tag="yT")
                y_sb = mid.tile([P, 2, D], mm_dt, tag="ysb")
                if last_ymajor and ib == BPC - 1:
                    # drain batch: y-major form [s, d] per expert — double the
                    # PE rows, but two fewer cross-engine hops in the chain
                    # (no yT copy, no transpose) while nothing else overlaps.
                    for e in range(E):
                        ec, po = e // 2, (e % 2) * S
                        y_ps = yT_ps[po : po + S, ec, 0:D]
                        for hc in range(4):
                            nc.tensor.matmul(
                                y_ps,
                                h_slice(hc, e),
                                w2_sb[:, e, hc, :],
                                start=(hc == 0),
                                stop=(hc == 3),
                            )
                        yse = "scalar" if e % 2 else "vector"
                        scaled_copy(yse, y_sb[po : po + S, ec, :], y_ps,
                                    recip_d[po : po + S, ec : ec + 1])
                        if has_b2:
                            nc.vector.tensor_add(
                                y_sb[po : po + S, ec, :],
                                y_sb[po : po + S, ec, :],
                                b2_tiles[g][po : po + S, ec, :],
                            )
                    return (exp_lT, recip_c, y_sb)

                # yT [d, es] = w2sel_e.T @ h_e per (expert, d-chunk): 64-wide
                # free dim — half the PE rows of the y-major form.
                yT_sb = mid.tile([P, 2, ES], y_dt, tag="yTs")

                def yt_copy(eh):
                    src = yt_src[eh]
                    if fine_last and ib >= BPC - fine_last:
                        split2([
                            (yT_sb[:, 0, ts(eh, 2 * S)], src[:, 0, :]),
                            (yT_sb[:, 1, ts(eh, 2 * S)], src[:, 1, :]),
                        ])
                        return
                    ye = ("scalar" if eh and last_par
                          and BPC - lp_lo <= ib < lp_hi else ytc_eng)
                    if ye == "scalar":
                        nc.scalar.activation(
                            yT_sb[:, :, ts(eh, 2 * S)], src, AF.Copy,
                        )
                    else:
                        eng(ye).tensor_copy(yT_sb[:, :, ts(eh, 2 * S)], src)

                if yt2_ph:
                    # the eh1 half accumulates in a ph-pool buffer (free
                    # after its ReLU) so its matmuls carry no PSUM-bank WAR
                    # against the eh0 copy
                    yT1_ps = psp.tile([P, 2, 2 * S], F32, tag="ph", bufs=2)
                yt_src = [yT_ps[:, :, 0 : 2 * S],
                          yT1_ps if yt2_ph else yT_ps[:, :, 2 * S : ES]]
                for eh in range(2):
                    dst = yt_src[eh]
                    for e2 in range(2):
                        e = 2 * eh + e2
                        for dc in range(2):
                            for hc in range(4):
                                nc.tensor.matmul(
                                    dst[:, dc, ds(e2 * S, S)],
                                    w2_sb[:, e, hc, ds(dc * P, P)],
                                    h_slice(hc, e),
                                    start=(hc == 0),
                                    stop=(hc == 3),
                                )
                    if not yt_single:
                        # per-half copy: earlier first transpose, but the eh1
                        # matmuls carry a PSUM-bank WAR on the eh0 copy
                        yt_copy(eh)
                if yt_single:
                    nc.vector.tensor_copy(yT_sb, yT_ps)

                # transpose yT -> y [es, d] (fp16 PSUM), scale rows by the
                # dispatch normalizer on the PSUM->SBUF copy. The PSUM tile
                # is padded to a full 2KiB bank so no other tile shares the
                # bank with PE transpose writes.
                # the transpose reads yT_sb (SBUF), which the yT copy wrote
                # after the yT bank's accumulation groups closed — so reusing
                # the yT bank for the transpose output adds no ordering the
                # data deps don't already impose, and frees a PSUM bank.
                ytr_ps = psp.tile(
                    [P, 4, D], y_dt, tag="yT" if ytr_share else "ytr"
                )
                for ec in range(2):
                    for dc in range(2):
                        nc.tensor.transpose(
                            ytr_ps[:, ec, ts(dc, P)],
                            yT_sb[:, dc, ts(ec, P)],
                            ident,
                        )
                    yse = ("scalar" if ec and last_par
                           and BPC - lp_lo <= ib < lp_hi
                           and not (endgame and ib == BPC - 1) else ysc_eng)
                    scaled_copy(yse, y_sb[:, ec, :], ytr_ps[:, ec, :],
                                recip_d[:, ec : ec + 1])
                    if has_b2:
                        nc.vector.tensor_add(
                            y_sb[:, ec, :], y_sb[:, ec, :],
                            b2_tiles[g][:, ec, :],
                        )
                return (exp_lT, recip_c, y_sb)

            def stage3(ib, ctx):
                exp_lT, recip_c, y_sb = ctx
                # out [m, d] = exp_lT.T @ y, then combine normalization.
                if ou_split:
                    # separate banks per m-half: the mc1 matmuls don't carry
                    # a PSUM-bank WAR on the mc0 scale
                    ou0_ps = psp.tile([P, D], F32, tag="ou")
                    ou1_ps = psp.tile([P, D], F32, tag="ou1")
                    ou_t = [ou0_ps, ou1_ps]
                else:
                    ou_ps = psp.tile([P, 2, D], F32, tag="ou")
                    ou_t = [ou_ps[:, 0, :], ou_ps[:, 1, :]]
                out_sb = io.tile([P, 2, D], o_dt, tag="out")
                ov = out_d[ib].rearrange("p (c d) -> p c d", c=2)
                oe = last_out_eng if ib == BPC - 1 else out_eng

                def ou_mm(mc):
                    for ec in range(2):
                        nc.tensor.matmul(
                            ou_t[mc],
                            exp_lT[:, ec, ts(mc, P)],
                            y_sb[:, ec, :],
                            start=(ec == 0),
                            stop=(ec == 1),
                        )

                def ou_scale_bcast():
                    rc = recip_c.broadcast_to([P, 2, D])
                    nc.vector.tensor_tensor(
                        out_sb, ou_ps, rc, mybir.AluOpType.mult
                    )

                def ou_scale(mc):
                    if fine_last and ib >= BPC - fine_last:
                        split2([
                            (out_sb[:, mc, 0:D // 2], ou_t[mc][:, 0:D // 2]),
                            (out_sb[:, mc, D // 2 :], ou_t[mc][:, D // 2 :]),
                        ], scale=recip_c[:, mc : mc + 1])
                    else:
                        scaled_copy(oe[mc], out_sb[:, mc, :], ou_t[mc],
                                    recip_c[:, mc : mc + 1])
                    if ib == BPC - 1:
                        # last batch: SP queue is empty; ship each half as
                        # soon as its scale finishes
                        nc.sync.dma_start(out=ov[:, mc, :], in_=out_sb[:, mc, :])

                if (endgame == 3 and ib == BPC - 1) or (
                    endgame == 4 and ib >= BPC - 2
                ):
                    # endgame v3: both accumulation groups close, then ONE
                    # broadcast tensor_tensor multiply scales both halves in
                    # a single DVE op, and one full store pays a single
                    # HWDGE pass.
                    ou_mm(0)
                    ou_mm(1)
                    rc = recip_c.broadcast_to([P, 2, D])
                    nc.vector.tensor_tensor(
                        out_sb, ou_ps, rc, mybir.AluOpType.mult
                    )
                    if ib == BPC - 1:
                        nc.sync.dma_start(out=out_d[ib], in_=out_sb)
                    else:
                        eng(store_eng).dma_start(out=out_d[ib], in_=out_sb)
                    return
                if endgame and ib == BPC - 1:
                    # endgame: both accumulation groups close, the two scales
                    # run on DVE and ACT in parallel, and ONE full store pays
                    # a single HWDGE pass (two serial 625ns passes cost more
                    # than the wider transfer).
                    ou_mm(0)
                    ou_mm(1)
                    scaled_copy(last_out_eng[0], out_sb[:, 0, :], ou_t[0],
                                recip_c[:, 0:1])
                    if endgame == 2:
                        # per-half stores: mc0's HWDGE pass overlaps the
                        # mc1 scale
                        nc.sync.dma_start(out=ov[:, 0, :], in_=out_sb[:, 0, :])
                        scaled_copy(last_out_eng[1], out_sb[:, 1, :], ou_t[1],
                                    recip_c[:, 1:2])
                        nc.sync.dma_start(out=ov[:, 1, :], in_=out_sb[:, 1, :])
                    else:
                        scaled_copy(last_out_eng[1], out_sb[:, 1, :], ou_t[1],
                                    recip_c[:, 1:2])
                        nc.sync.dma_start(out=out_d[ib], in_=out_sb)
                    return
                if ou_order == "bcast":
                    ou_mm(0)
                    ou_mm(1)
                    ou_scale_bcast()
                    if ib == BPC - 1:
                        nc.sync.dma_start(out=out_d[ib], in_=out_sb)
                    else:
                        eng(store_eng).dma_start(out=out_d[ib], in_=out_sb)
                    return
                scale_after = ou_order == "scale_after" or (
                    ou_order == "last2" and ib >= BPC - 2
                )
                if not scale_after:
                    for mc in range(2):
                        ou_mm(mc)
                        ou_scale(mc)
                else:
                    # close both accumulation groups before any scale reads
                    # the bank: no WAR stall on the PE (pays off in the drain
                    # where no other batch fills the wait)
                    ou_mm(0)
                    ou_mm(1)
                    ou_scale(0)
                    ou_scale(1)
                if ib != BPC - 1:
                    # Stores ride the ACT HWDGE ring: on SP they would sit in
                    # the FIFO ahead of the next batch's weight loads and
                    # head-of-line block them.
                    eng(store_eng).dma_start(out=out_d[ib], in_=out_sb)

            # Software pipeline: stage1(ib) runs `lookahead` batches ahead of
            # stage2; stage3 trails stage2 by `s3_lag`. Each stage uses its
            # own PSUM tags so depth costs no extra PSUM.
            c1, c2 = {}, {}

            def emit_s2(j):
                c2[j] = stage2(j, c1.pop(j))

            def emit_s3(j):
                stage3(j, c2.pop(j))

            for ib in range(BPC):
                c1[ib] = stage1(ib)
                j = ib - lookahead
                if j >= 0:
                    emit_s2(j)
                if j - s3_lag >= 0:
                    emit_s3(j - s3_lag)
            for j in range(max(0, BPC - lookahead), BPC):
                emit_s2(j)
                if j - s3_lag >= 0 and j - s3_lag in c2:
                    emit_s3(j - s3_lag)
            for j in sorted(c2):
                stage3(j, c2[j])

    nc.compile()
    return nc


class _Runner:
    """Compile once per process; re-execute via a cached jitted shard_map."""

    def __init__(self, pattern, mm_dt=F32, y_dt=None, has_b2=False):
        # The Tile PSUM slot allocator is heuristic and can spuriously fail
        # near capacity; retry a few times.
        kw = {}
        if pattern == (4, 2, 1, 1):
            # hand-tuned load schedule for the common pattern (43590 ns vs
            # 43610 with the generic rule)
            kw["w2_load_at"] = (2, 4, 5, 5)
        last = None
        for _ in range(4):
            try:
                self.nc = build_nc(
                    pattern, mm_dt=mm_dt, y_dt=y_dt, has_b2=has_b2, **kw
                )
                break
            except ValueError as e:
                last = e
        else:
            raise last
        self.has_b2 = has_b2
        self._fn = None

    def _build_fn(self):
        import jax
        from jax.sharding import Mesh, PartitionSpec
        from jax.experimental.shard_map import shard_map
        from concourse import bass2jax
        from concourse.bass2jax import _bass_exec_p, partition_id_tensor

        bass2jax.install_neuronx_cc_hook()
        nc = self.nc
        partition_name = (
            nc.partition_id_tensor.name if nc.partition_id_tensor else None
        )
        in_names, out_names, out_avals, zero_outs = [], [], [], []
        for alloc in nc.m.functions[0].allocations:
            if not isinstance(alloc, mybir.MemoryLocationSet):
                continue
            name = alloc.memorylocations[0].name
            if alloc.kind == "ExternalInput":
                if name != partition_name:
                    in_names.append(name)
            elif alloc.kind == "ExternalOutput":
                shape = tuple(alloc.tensor_shape)
                dtype = mybir.dt.np(alloc.dtype)
                out_names.append(name)
                out_avals.append(jax.core.ShapedArray(shape, dtype))
                zero_outs.append(np.zeros(shape, dtype))
        n_params = len(in_names)
        all_in_names = list(in_names) + list(out_names)
        if partition_name is not None:
            all_in_names.append(partition_name)

        def _body(*args):
            operands = list(args)
            if partition_name is not None:
                operands.append(partition_id_tensor())
            outs = _bass_exec_p.bind(
                *operands,
                out_avals=tuple(out_avals),
                in_names=tuple(all_in_names),
                out_names=tuple(out_names),
                lowering_input_output_aliases=(),
                sim_require_finite=True,
                sim_require_nnan=True,
                nc=nc,
            )
            return tuple(outs)

        devices = jax.devices()[:N_CORES]
        assert len(devices) >= N_CORES, (
            f"need {N_CORES} NeuronCores, found {len(jax.devices())}"
        )
        mesh = Mesh(np.asarray(devices), ("core",))
        n_outs = len(out_names)
        sharded = jax.jit(
            shard_map(
                _body,
                mesh=mesh,
                in_specs=(PartitionSpec("core"),) * (n_params + n_outs),
                out_specs=(PartitionSpec("core"),) * n_outs,
                check_rep=False,
            ),
            donate_argnums=tuple(range(n_params, n_params + n_outs)),
            keep_unused=True,
        )
        self._in_names = in_names
        self._out_names = out_names
        self._out_avals = out_avals
        self._zero_outs = zero_outs
        self._fn = sharded

    def run(self, in_maps):
        """in_maps: list of N_CORES dicts name->np.ndarray. Returns per-core
        dict of outputs."""
        if self._fn is None:
            self._build_fn()
        concat_in = [
            np.concatenate([in_maps[c][nm] for c in range(N_CORES)], axis=0)
            for nm in self._in_names
        ]
        concat_zeros = [
            np.zeros((N_CORES * z.shape[0], *z.shape[1:]), z.dtype)
            for z in self._zero_outs
        ]
        out_arrs = self._fn(*concat_in, *concat_zeros)
        return [
            {
                nm: np.asarray(out_arrs[i]).reshape(
                    N_CORES, *self._out_avals[i].shape
                )[c]
                for i, nm in enumerate(self._out_names)
            }
            for c in range(N_CORES)
        ]


_runner_cache = {}


def _prep_inputs(obs, action, phi, w1, b1, w2, b2):
    obs = np.ascontiguousarray(np.asarray(obs, dtype=np.float32))
    action = np.asarray(action).astype(np.int64)
    phi = np.asarray(phi, dtype=np.float32)
    w1 = np.ascontiguousarray(np.asarray(w1, dtype=np.float32))
    b1 = np.asarray(b1, dtype=np.float32)
    w2 = np.asarray(w2, dtype=np.float32)
    b2 = np.asarray(b2, dtype=np.float32)
    if np.any(b1):
        # The device kernel folds the dispatch-softmax normalizer past the
        # ReLU, which requires b1 == 0 (true for this problem's inputs).
        # Any other input falls back to an exact host computation.
        return None
    pattern, order = solve_groups(action)
    G = len(pattern)
    obs = obs[order]
    action_s = action[order]
    # group-leader action per (core, group)
    b2g = []
    for g, s in enumerate(pattern):
        b2g += [g] * s
    grp_action = np.empty((N_CORES, G), np.int64)
    for c in range(N_CORES):
        for ib in range(BPC):
            grp_action[c, b2g[ib]] = action_s[c * BPC + ib]

    obsT = obs.transpose(0, 2, 1)
    # obs [B,M,D] -> (b, p, mc, d): m = mc*128 + p ; obsT -> (b, p, dc, m)
    obs_k = obs.reshape(B, 2, P, D).transpose(0, 2, 1, 3)
    obsT_k = obsT.reshape(B, 2, P, M).transpose(0, 2, 1, 3)
    oo_k = np.ascontiguousarray(
        np.concatenate([obs_k, obsT_k], axis=2)
    ).reshape(B, P, 4 * D)
    # phi [D,ES] -> (p, dc, es)
    phi_k = np.ascontiguousarray(
        phi.reshape(2, P, ES).transpose(1, 0, 2)
    ).reshape(P, 2 * ES)
    # w1 [E,D,H] -> (p, dc, e, h)
    w1_k = np.ascontiguousarray(
        w1.reshape(E, 2, P, H).transpose(2, 1, 0, 3)
    ).reshape(P, 2 * E * H)
    # per-group action-selected slices: w2grp [NC, G, P, E*4*D]
    w2r = w2.reshape(E, H, A, D)
    w2sel = w2r[:, :, grp_action.reshape(-1), :].transpose(2, 0, 1, 3)
    w2_k = np.ascontiguousarray(
        w2sel.reshape(N_CORES * G, E, 4, P, D).transpose(0, 3, 1, 2, 4)
    ).reshape(N_CORES, G, P, E * 4 * D)
    has_b2 = bool(np.any(b2))
    b2_k = None
    if has_b2:
        b2r = b2.reshape(E, A, D)
        b2_k = np.ascontiguousarray(
            b2r[:, grp_action.reshape(-1), :].transpose(1, 0, 2)
        ).reshape(N_CORES, G, 1, E * D)

    np_main = mybir.dt.np(MM_DT)
    np_y = mybir.dt.np(Y_DT)
    oo_k = oo_k.astype(np_main)
    phi_k = phi_k.astype(np_main)
    w1_k = w1_k.astype(np_main)
    w2_k = w2_k.astype(np_y)
    if has_b2:
        b2_k = b2_k.astype(np_y)
    in_maps = []
    for c in range(N_CORES):
        sl = slice(c * BPC, (c + 1) * BPC)
        m = {
            "oo": oo_k[sl],
            "phi": phi_k,
            "w1": w1_k,
            "w2grp": w2_k[c],
        }
        if has_b2:
            m["b2grp"] = b2_k[c]
        in_maps.append(m)
    return in_maps, has_b2, order, pattern


def get_runner(has_b2, pattern=None, mm_dt=None, y_dt=None):
    if pattern is None:
        pattern = PATTERNS[0]
    if mm_dt is None:
        mm_dt = MM_DT
    if y_dt is None:
        y_dt = Y_DT
    key = (pattern, str(mm_dt), str(y_dt), has_b2)
    if key not in _runner_cache:
        _runner_cache[key] = _Runner(
            pattern, mm_dt=mm_dt, y_dt=y_dt, has_b2=has_b2
        )
    return _runner_cache[key]


def _numpy_reference(obs, action, phi, w1, b1, w2, b2):
    obs = np.asarray(obs, np.float64)
    logits = np.einsum("bmd,des->bmes", obs, np.asarray(phi, np.float64).reshape(D, E, S))
    lmax = logits.max(axis=1, keepdims=True)
    el = np.exp(logits - lmax)
    dispatch = el / el.sum(axis=1, keepdims=True)
    lf = logits.reshape(B, M, E * S)
    ec_ = np.exp(lf - lf.max(axis=-1, keepdims=True))
    combine = (ec_ / ec_.sum(axis=-1, keepdims=True)).reshape(B, M, E, S)
    slots = np.einsum("bmd,bmes->besd", obs, dispatch)
    h = np.maximum(
        np.einsum("besd,edh->besh", slots, np.asarray(w1, np.float64))
        + np.asarray(b1, np.float64)[None, :, None, :], 0
    )
    y = np.einsum("besh,ehk->besk", h, np.asarray(w2, np.float64)) + np.asarray(
        b2, np.float64
    )[None, :, None, :]
    out = np.einsum("bmes,besk->bmk", combine, y)
    out = out.reshape(B, M, A, D).transpose(0, 2, 1, 3)
    oh = np.eye(A)[np.asarray(action).astype(np.int64)]
    return np.einsum("bamd,ba->bmd", out, oh).astype(np.float32)


def kernel(obs, action, phi, w1, b1, w2, b2):
    prep = _prep_inputs(obs, action, phi, w1, b1, w2, b2)
    if prep is None:
        return _numpy_reference(obs, action, phi, w1, b1, w2, b2)
    in_maps, has_b2, order, pattern = prep
    runner = get_runner(has_b2, pattern)
    out_k = None
    last_err = None
    for attempt in range(4):
        try:
            results = runner.run(in_maps)
        except Exception as e:  # transient device wedges recover on retry
            last_err = e
            time.sleep(2.0)
            continue
        cand = np.concatenate(
            [results[c]["out"] for c in range(N_CORES)], axis=0
        )
        out_k = cand
        # transient device glitches can yield non-finite garbage without
        # raising; for these inputs the true output is always finite, so
        # re-run rather than return it
        if np.isfinite(cand.astype(np.float32)).all():
            break
    if out_k is None:
        raise last_err
    # (b, p, mc, d) -> [B, M, D] with m = mc*128 + p; undo the action sort
    out_s = (
        out_k.astype(np.float32)
        .reshape(B, P, 2, D)
        .transpose(0, 2, 1, 3)
        .reshape(B, M, D)
    )
    out = np.empty_like(out_s)
    out[order] = out_s
    return np.ascontiguousarray(out)


# revision 58
# speedup vs baseline: 1.0004x; 1.0004x over previous
"""Soft-MoE discrete-action transition network — Trainium2 Bass kernel.

Problem shapes (hardcoded):
  obs [B=64, M=256, D=256] f32, action [B=64] i64,
  phi [D, E=4, S=64] f32, w1 [E, D, H=512] f32, b1 [E, H] f32 (zeros),
  w2 [E, H, A*D=4608] f32, b2 [E, A*D] f32 (zeros).  Output [B, M, D] f32.

Strategy (v2):
  * Host gathers the action-selected slice of w2 per batch
    (w2sel[b] = w2[:, :, a_b*D:(a_b+1)*D]) — the one-hot contraction at the
    end of the reference selects exactly one D-wide block per batch.
  * Data-parallel over batch: 8 batch elements per NeuronCore. Batches with
    EQUAL actions are grouped so one w2sel tile serves the whole group: the
    host solves for a static per-core group pattern (e.g. (4,2,1,1) = one
    quad + one pair + two singles = 4 w2 tiles/core instead of 8), halving
    the dominant DMA traffic. The pattern is compiled into the program; the
    host picks the densest feasible pattern for the actual action multiset.
  * obs+obsT ride one DMA per batch; output is stored fp16 (converted back
    to f32 on host). All layout rearrangement happens on the host so every
    device DMA is a contiguous [128, N] copy.
  * Matmul operands are fp16; PSUM accumulation and softmax plumbing fp32.
  * Per batch, on device (P=128 partition chunks):
      logits  [m,es] = obsT.T @ phi      (lhsT=obsT[d,m], rhs=phi[d,es])
      logitsT [es,m] = phi.T  @ obsT     (same operands)
      exp both (ScalarE; accum_out yields both softmax denominators free)
      slotsT  [d,es] = obs.T @ exp_l     (unnormalized dispatch)
      pre_h   [h,es] = w1_e.T @ slotsT   per expert; ReLU (dispatch softmax
                        normalizer folded past ReLU — valid since b1 == 0;
                        nonzero b1 falls back to an exact host computation)
      yT      [d,es] = w2sel_e.T @ h_e   per (expert, d-chunk): free dim 64
                        instead of 256 — half the PE rows of the y-major
                        form; then PE-transpose (4x 128x128 fp16) to y[es,d]
                        and scale rows by 1/colsum (dispatch) on the
                        PSUM->SBUF copy.
      out     [m,d]  = exp_lT.T @ y; scale rows by 1/rowsum (combine)
  * Schedule (tuned against the TimelineSim cost model, 52.7us -> 43.6us):
    - 3-stage software pipeline (stage1 logits/dispatch, stage2 expert MLP,
      stage3 combine/store) with stage1 running 3 batches ahead; each stage
      owns its PSUM tags so depth costs no extra PSUM (8 banks exactly).
    - A tiny PE warm-up (5 identity transposes off the const identity tile)
      starts the cost model's p-state ramp clock during the first DMAs, so
      every real matmul runs at the full 2.4 GHz rate.
    - w2 group loads are scheduled between the oo loads (not before batch
      2, not after their consumer's emission); w1 loads after oo1.
    - PSUM->SBUF traffic is balanced across DVE and ACT (ReLU half on each;
      exps on ACT; slots/yT/y/out copies on DVE); mid-stream output stores
      issue from the idle GPSIMD queue (SWDGE) to keep HWDGE free; the last
      batch stores per-half from SP and splits its copies across engines.
"""

import os
import sys
import time

import numpy as np

for _p in ("/opt/trn_rl_repo",):
    if os.path.isdir(_p) and _p not in sys.path:
        sys.path.append(_p)

import concourse.bass as bass
import concourse.mybir as mybir
import concourse.tile as tile
from concourse import bacc
from concourse.bass import ds, ts
from concourse.masks import make_identity

B, M, D, A = 64, 256, 256, 18
E, S, H = 4, 64, 512
ES = E * S
N_CORES = 8
BPC = B // N_CORES  # batches per core
P = 128
F32 = mybir.dt.float32
F16 = mybir.dt.float16

AF = mybir.ActivationFunctionType

MM_DT = getattr(mybir.dt, os.environ.get("MOE_MM_DT", "float16"))
Y_DT = getattr(mybir.dt, os.environ.get("MOE_Y_DT", "float16"))

# Candidate per-core group patterns, densest first. Each tuple sums to BPC;
# all 8 cores share the pattern, so a pattern needs 8 equal-action groups of
# every size >= 2 to exist in the global action multiset.
PATTERNS = [
    (4, 2, 1, 1), (3, 3, 1, 1), (3, 2, 2, 1), (2, 2, 2, 2), (5, 2, 1),
    (4, 2, 2), (4, 3, 1), (3, 3, 2), (2, 2, 2, 1, 1), (3, 2, 1, 1, 1),
    (4, 1, 1, 1, 1), (2, 2, 1, 1, 1, 1), (3, 1, 1, 1, 1, 1),
    (2, 1, 1, 1, 1, 1, 1), (1, 1, 1, 1, 1, 1, 1, 1),
]


def solve_groups(action):
    """Pick the densest feasible pattern; return (pattern, order) where
    order[c*BPC + i] = original batch index placed at core c, slot i.
    Slots are laid out group-by-group following the pattern."""
    from collections import Counter

    by_action = {}
    for idx, a in enumerate(np.asarray(action).tolist()):
        by_action.setdefault(a, []).append(idx)

    for pat in PATTERNS:
        cnt = Counter({a: len(v) for a, v in by_action.items()})
        need = sorted((s for s in pat for _ in range(N_CORES)), reverse=True)
        picks = []  # action per group, aligned with `need`
        ok = True
        for s in need:
            if s == 1:
                picks.append(None)  # singles take leftovers later
                continue
            a = max(cnt, key=lambda k: cnt[k])
            if cnt[a] < s:
                ok = False
                break
            cnt[a] -= s
            if cnt[a] == 0:
                del cnt[a]
            picks.append(a)
        if not ok:
            continue
        # materialize: pools of remaining batch indices per action
        pools = {a: list(v) for a, v in by_action.items()}
        groups_by_size = {}
        for s, a in zip(need, picks):
            if s == 1:
                continue
            groups_by_size.setdefault(s, []).append(pools[a][:s])
            pools[a] = pools[a][s:]
        leftovers = [i for v in pools.values() for i in v]
        li = 0
        order = []
        taken = {s: 0 for s in groups_by_size}
        for c in range(N_CORES):
            for s in pat:
                if s == 1:
                    order.append(leftovers[li])
                    li += 1
                else:
                    g = groups_by_size[s][taken[s]]
                    taken[s] += 1
                    order.extend(g)
        assert li == len(leftovers) and len(order) == B
        return pat, np.asarray(order)
    raise AssertionError("(1,)*BPC is always feasible")


def build_nc(pattern, mm_dt=F32, y_dt=None, has_b2=False, *, o_dt=F16,
             io_bufs=4, mid_bufs=3, w1_late=True,
             relu_eng=("vector", "scalar"), slots_eng="vector",
             ytc_eng="vector", ysc_eng="vector", out_eng=("vector", "vector"),
             last_out_eng=("vector", "vector"), store_eng="gpsimd",
             lookahead=3, warmup=5, w1_at=1, w2_load_at=None,
             split_first=False, last_par=True, last_ymajor=False, s3_lag=1,
             s1_order="interleaved", sl_share=False, ou_split=False,
             ou_order="interleave", yt_single=False, ytr_share=False,
             w1_split=False, fine_last=0, endgame=3, conv=(), yt2_ph=False,
             lp_lo=2, lp_hi=BPC, w2_halves=0):
    """Build the per-core Bass program (one NeuronCore, BPC batches).

    pattern: per-core group sizes, e.g. (4,2,1,1). Batch ib belongs to group
    g per the cumulative pattern; one w2 tile is loaded per group.
    """
    if y_dt is None:
        y_dt = mm_dt
    G = len(pattern)
    # batch index -> group index, and group start flags
    b2g = []
    for g, s in enumerate(pattern):
        b2g += [g] * s
    assert len(b2g) == BPC
    if w2_load_at is None:
        # Each group's load rides a stage1 near (but not before emission of)
        # its first consumer: at least batch 2 (so the early oo loads are not
        # starved), at most first_batch + lookahead (so the load is EMITTED
        # before stage2(first_batch) consumes the tile).
        w2_load_at = tuple(
            min(max(b2g.index(g), 2), b2g.index(g) + lookahead, BPC - 1)
            for g in range(G)
        )
    assert len(w2_load_at) == G
    for g in range(G):
        assert w2_load_at[g] <= b2g.index(g) + lookahead, (
            "w2 load emitted after its consumer"
        )

    nc = bacc.Bacc("TRN2", target_bir_lowering=False, debug=False)

    oo_d = nc.dram_tensor(
        "oo", [BPC, P, 4 * D], mm_dt, kind="ExternalInput"
    ).ap()
    phi_d = nc.dram_tensor("phi", [P, 2 * ES], mm_dt, kind="ExternalInput").ap()
    w1_d = nc.dram_tensor("w1", [P, 2 * E * H], mm_dt, kind="ExternalInput").ap()
    w2_d = nc.dram_tensor(
        "w2grp", [G, P, E * 4 * D], y_dt, kind="ExternalInput"
    ).ap()
    if has_b2:
        b2_d = nc.dram_tensor(
            "b2grp", [G, 1, E * D], y_dt, kind="ExternalInput"
        ).ap()
    out_d = nc.dram_tensor("out", [BPC, P, 2 * D], o_dt, kind="ExternalOutput").ap()

    def eng(name):
        return {"vector": nc.vector, "scalar": nc.scalar,
                "gpsimd": nc.gpsimd, "sync": nc.sync}[name]

    def scaled_copy(engname, out, in0, scale):
        if engname == "scalar":
            nc.scalar.activation(out, in0, AF.Copy, scale=scale)
        else:
            eng(engname).tensor_scalar_mul(out, in0=in0, scalar1=scale)

    def relu_op(engname, out, in0):
        if engname == "scalar":
            nc.scalar.activation(out, in0, AF.Relu)
        else:
            eng(engname).tensor_scalar_max(out, in0, 0.0)

    def split2(pairs, scale=None):
        """pairs: [(out0, in0), (out1, in1)] — half on DVE, half on ACT,
        in parallel, to halve the chain latency of a drain-batch copy."""
        (o0, i0), (o1, i1) = pairs
        if scale is None:
            nc.vector.tensor_copy(o0, i0)
            nc.scalar.activation(o1, i1, AF.Copy)
        else:
            nc.vector.tensor_scalar_mul(o0, in0=i0, scalar1=scale)
            nc.scalar.activation(o1, i1, AF.Copy, scale=scale)

    def split2_relu(pairs):
        (o0, i0), (o1, i1) = pairs
        nc.vector.tensor_scalar_max(o0, i0, 0.0)
        nc.scalar.activation(o1, i1, AF.Relu)

    with tile.TileContext(nc) as tc:
        with (
            tc.tile_pool(name="const", bufs=1) as const,
            tc.tile_pool(name="io", bufs=io_bufs) as io,
            tc.tile_pool(name="mid", bufs=mid_bufs) as mid,
            tc.tile_pool(name="psum", bufs=1, space="PSUM") as psp,
        ):
            phi_sb = const.tile([P, 2, ES], mm_dt)
            if split_first:
                # split the phi load per d-chunk so the first logits matmul
                # (needing only dc0 of phi+obsT) starts ~700ns earlier
                phi_v = phi_d.rearrange("p (c s) -> p c s", c=2)
                for dc in range(2):
                    nc.sync.dma_start(out=phi_sb[:, dc, :], in_=phi_v[:, dc, :])
            else:
                nc.sync.dma_start(out=phi_sb, in_=phi_d)
            ident = const.tile([P, P], y_dt)
            make_identity(nc, ident)
            if warmup:
                # PE p-state warm-up: the cost model runs matmuls at half
                # speed until the engine has been continuously busy for 3us.
                # A stream of identity transposes into scratch PSUM (same tag
                # as the y transposes, so no extra PSUM) ramps the PE while
                # the first DMAs are in flight.
                warm_ps = psp.tile(
                    [P, 4, D], y_dt, tag="yT" if ytr_share else "ytr"
                )
                for _ in range(warmup):
                    nc.tensor.transpose(warm_ps[:, 0, 0:P], ident, ident)
            w1_sb = const.tile([P, 2, E, H], mm_dt)
            if not w1_late:
                nc.sync.dma_start(out=w1_sb, in_=w1_d)
            w2_tiles = [
                const.tile([P, E, 4, D], y_dt, name=f"w2t{g}") for g in range(G)
            ]
            if has_b2:
                b2_tiles = [
                    const.tile([P, 2, D], mm_dt, name=f"b2t{g}") for g in range(G)
                ]

            def stage1(ib):
                # obs and obsT ride one DMA; host stores them adjacently
                oo_sb = io.tile([P, 4, D], mm_dt, tag="oo")
                oo_v = oo_d[ib].rearrange("p (c d) -> p c d", c=4)
                if ib in conv:
                    # early batches: the serial DMA engine is the pacer while
                    # PE/DVE idle, so load only obs (half the bytes) and
                    # build obsT on-device with PE transposes
                    nc.sync.dma_start(out=oo_sb[:, 0:2, :], in_=oo_v[:, 0:2, :])
                    obs_sb = oo_sb[:, 0:2, :]
                    obsT_ps = psp.tile([P, 4, D], y_dt, tag="ytr")
                    for mc in range(2):
                        for dc in range(2):
                            nc.tensor.transpose(
                                obsT_ps[:, dc, ts(mc, P)],
                                obs_sb[:, mc, ts(dc, P)],
                                ident,
                            )
                    obsT_sb = oo_sb[:, 2:4, :]
                    for dc in range(2):
                        nc.vector.tensor_copy(
                            obsT_sb[:, dc, :], obsT_ps[:, dc, :]
                        )
                elif ib == 0 and split_first:
                    # obsT half first (gates the first logits matmuls),
                    # obs half after (only needed by the slots matmuls)
                    nc.sync.dma_start(out=oo_sb[:, 2:4, :], in_=oo_v[:, 2:4, :])
                    nc.sync.dma_start(out=oo_sb[:, 0:2, :], in_=oo_v[:, 0:2, :])
                    obs_sb = oo_sb[:, 0:2, :]
                    obsT_sb = oo_sb[:, 2:4, :]
                else:
                    nc.sync.dma_start(out=oo_sb, in_=oo_v)
                    obs_sb = oo_sb[:, 0:2, :]
                    obsT_sb = oo_sb[:, 2:4, :]
                if w1_late and ib in (
                    (w1_at, w1_at + 1) if w1_split else (w1_at,)
                ):
                    # logits only need phi+obsT; deferring the w1 const load
                    # past the first oo loads lets PE start earlier. With
                    # w1_split the h-halves arrive separately so the first
                    # preh chunks can start off the first half.
                    w1_v = w1_d.rearrange("p (c e h) -> p c e h", c=2, e=E)
                    if not w1_split:
                        nc.sync.dma_start(out=w1_sb, in_=w1_d)
                    else:
                        hh = ib - w1_at
                        nc.sync.dma_start(
                            out=w1_sb[:, :, :, ds(hh * H // 2, H // 2)],
                            in_=w1_v[:, :, :, ds(hh * H // 2, H // 2)],
                        )
                g = b2g[ib]
                # w2 tiles are independent const tiles, so their loads can be
                # scheduled freely (w2_load_at[g] = stage1 index issuing the
                # load) — keeps the late groups' loads off the pipeline-drain
                # critical path without starving the early oo loads.
                for g_ in [gi for gi, lb in enumerate(w2_load_at) if lb == ib]:
                    w2v = w2_d[g_].rearrange("p (e k) -> p e k", e=E)
                    if g_ < w2_halves:
                        # split into expert-halves so the eh0 experts land
                        # ~1.5us earlier than the monolithic transfer
                        nc.sync.dma_start(
                            out=w2_tiles[g_][:, 0:2], in_=w2v[:, 0:2])
                        nc.sync.dma_start(
                            out=w2_tiles[g_][:, 2:4], in_=w2v[:, 2:4])
                    else:
                        nc.sync.dma_start(out=w2_tiles[g_], in_=w2v)
                    if has_b2:
                        # broadcast b2grp[g][e] across the 64 slot partitions
                        # of each expert (pg = e % 2): 0-stride partition DMAs
                        for pg in range(2):
                            srcap = bass.AP(
                                tensor=b2_d.tensor,
                                offset=g_ * E * D + pg * D,
                                ap=[[0, S], [2 * D, 2], [1, D]],
                            )
                            nc.sync.dma_start(
                                out=b2_tiles[g_][pg * S : (pg + 1) * S, :, :],
                                in_=srcap,
                            )

                # logits [m, es] and logitsT [es, m], chunk-interleaved so
                # the first exp (and the slot matmuls) start earlier
                lg_ps = psp.tile([P, 2, ES], F32, tag="lg")
                lgT_ps = psp.tile([P, 2, M], F32, tag="lgT")
                exp_l = mid.tile([P, 2, ES], mm_dt, tag="expl")
                exp_lT = mid.tile([P, 2, M], mm_dt, tag="explT")
                sums = mid.tile([P, 4], F32, tag="sums")
                def lg_mm(c):
                    for dc in range(2):
                        nc.tensor.matmul(
                            lg_ps[:, c, :],
                            obsT_sb[:, dc, ts(c, P)],
                            phi_sb[:, dc, :],
                            start=(dc == 0),
                            stop=(dc == 1),
                        )

                def lgT_mm(c):
                    for dc in range(2):
                        nc.tensor.matmul(
                            lgT_ps[:, c, :],
                            phi_sb[:, dc, ts(c, P)],
                            obsT_sb[:, dc, :],
                            start=(dc == 0),
                            stop=(dc == 1),
                        )

                def exp_op(c):
                    nc.scalar.activation(
                        exp_l[:, c, :], lg_ps[:, c, :], AF.Exp,
                        accum_out=sums[:, c : c + 1],
                    )

                def expT_op(c):
                    nc.scalar.activation(
                        exp_lT[:, c, :], lgT_ps[:, c, :], AF.Exp,
                        accum_out=sums[:, 2 + c : 3 + c],
                    )

                if s1_order == "interleaved" or (
                    s1_order == "mm_first01" and ib > 1
                ):
                    # first exp starts earliest, but the c1 logits matmuls
                    # carry a PSUM-bank WAR on the c0 exps
                    for c in range(2):
                        lg_mm(c)
                        exp_op(c)
                        lgT_mm(c)
                        expT_op(c)
                elif s1_order == "lg_first":
                    # exp_l (gating the slots matmuls) first; exp_lT is not
                    # consumed until stage3, so its chain can trail
                    lg_mm(0)
                    exp_op(0)
                    lg_mm(1)
                    exp_op(1)
                    lgT_mm(0)
                    expT_op(0)
                    lgT_mm(1)
                    expT_op(1)
                else:
                    # all four matmul groups close their PSUM banks before
                    # any exp reads them: no WAR stalls on the PE
                    lg_mm(0)
                    lg_mm(1)
                    lgT_mm(0)
                    exp_op(0)
                    lgT_mm(1)
                    exp_op(1)
                    expT_op(0)
                    expT_op(1)

                # one reciprocal for both softmax denominators:
                # cols 0-1 = combine (per m-chunk), cols 2-3 = dispatch
                recips = mid.tile([P, 4], F32, tag="recips")
                nc.vector.reciprocal(recips, sums)
                recip_c = recips[:, 0:2]
                recip_d = recips[:, 2:4]

                # slotsT [d, es] = obs.T @ exp_l (unnormalized dispatch)
                sl_ps = psp.tile([P, 2, ES], F32, tag="lg" if sl_share else "sl")
                for dc in range(2):
                    for mc in range(2):
                        nc.tensor.matmul(
                            sl_ps[:, dc, :],
                            obs_sb[:, mc, ts(dc, P)],
                            exp_l[:, mc, :],
                            start=(mc == 0),
                            stop=(mc == 1),
                        )
                slots_sb = mid.tile([P, 2, ES], mm_dt, tag="slots")
                for eh in range(2):
                    if fine_last and ib >= BPC - fine_last:
                        # drain batches: halve the copy latency by running
                        # the dc halves on DVE and ACT in parallel
                        split2([
                            (slots_sb[:, 0, ts(eh, 2 * S)],
                             sl_ps[:, 0, ts(eh, 2 * S)]),
                            (slots_sb[:, 1, ts(eh, 2 * S)],
                             sl_ps[:, 1, ts(eh, 2 * S)]),
                        ])
                        continue
                    se = ("scalar" if eh and last_par
                          and BPC - lp_lo <= ib < lp_hi else slots_eng)
                    if se == "scalar":
                        nc.scalar.activation(
                            slots_sb[:, :, ts(eh, 2 * S)],
                            sl_ps[:, :, ts(eh, 2 * S)], AF.Copy,
                        )
                    else:
                        eng(se).tensor_copy(
                            slots_sb[:, :, ts(eh, 2 * S)],
                            sl_ps[:, :, ts(eh, 2 * S)],
                        )

                return (slots_sb, exp_lT, recip_c, recip_d, g)

            def stage2(ib, ctx):
                slots_sb, exp_lT, recip_c, recip_d, g = ctx
                w2_sb = w2_tiles[g]
                # pre_h [h, (e,s)]: h laid out [p, eh, hc, 2S] so each es-half
                # (2 experts) is an independent pipeline — its yT matmuls
                # start after its own ReLU, not after all experts' pre_h.
                h_sb = mid.tile([P, 2, 4, 2 * S], y_dt, tag="h")
                for eh in range(2):
                    ph_ps = psp.tile([P, 4, 2 * S], F32, tag="ph", bufs=2)
                    for hc in range(4):
                        for e2 in range(2):
                            e = 2 * eh + e2
                            for dc in range(2):
                                nc.tensor.matmul(
                                    ph_ps[:, hc, ds(e2 * S, S)],
                                    w1_sb[:, dc, e, ts(hc, P)],
                                    slots_sb[:, dc, ds(e * S, S)],
                                    start=(dc == 0),
                                    stop=(dc == 1),
                                )
                    if fine_last and ib >= BPC - fine_last:
                        split2_relu([
                            (h_sb[:, eh, 0:2], ph_ps[:, 0:2]),
                            (h_sb[:, eh, 2:4], ph_ps[:, 2:4]),
                        ])
                    else:
                        relu_op(relu_eng[eh], h_sb[:, eh], ph_ps)

                def h_slice(hc, e):
                    return h_sb[:, e // 2, hc, ds((e % 2) * S, S)]

                yT_ps = psp.tile([P, 2, ES], F32, tag="yT")
                y_sb = mid.tile([P, 2, D], mm_dt, tag="ysb")
                if last_ymajor and ib == BPC - 1:
                    # drain batch: y-major form [s, d] per expert — double the
                    # PE rows, but two fewer cross-engine hops in the chain
                    # (no yT copy, no transpose) while nothing else overlaps.
                    for e in range(E):
                        ec, po = e // 2, (e % 2) * S
                        y_ps = yT_ps[po : po + S, ec, 0:D]
                        for hc in range(4):
                            nc.tensor.matmul(
                                y_ps,
                                h_slice(hc, e),
                                w2_sb[:, e, hc, :],
                                start=(hc == 0),
                                stop=(hc == 3),
                            )
                        yse = "scalar" if e % 2 else "vector"
                        scaled_copy(yse, y_sb[po : po + S, ec, :], y_ps,
                                    recip_d[po : po + S, ec : ec + 1])
                        if has_b2:
                            nc.vector.tensor_add(
                                y_sb[po : po + S, ec, :],
                                y_sb[po : po + S, ec, :],
                                b2_tiles[g][po : po + S, ec, :],
                            )
                    return (exp_lT, recip_c, y_sb)

                # yT [d, es] = w2sel_e.T @ h_e per (expert, d-chunk): 64-wide
                # free dim — half the PE rows of the y-major form.
                yT_sb = mid.tile([P, 2, ES], y_dt, tag="yTs")

                def yt_copy(eh):
                    src = yt_src[eh]
                    if fine_last and ib >= BPC - fine_last:
                        split2([
                            (yT_sb[:, 0, ts(eh, 2 * S)], src[:, 0, :]),
                            (yT_sb[:, 1, ts(eh, 2 * S)], src[:, 1, :]),
                        ])
                        return
                    ye = ("scalar" if eh and last_par
                          and BPC - lp_lo <= ib < lp_hi else ytc_eng)
                    if ye == "scalar":
                        nc.scalar.activation(
                            yT_sb[:, :, ts(eh, 2 * S)], src, AF.Copy,
                        )
                    else:
                        eng(ye).tensor_copy(yT_sb[:, :, ts(eh, 2 * S)], src)

                if yt2_ph:
                    # the eh1 half accumulates in a ph-pool buffer (free
                    # after its ReLU) so its matmuls carry no PSUM-bank WAR
                    # against the eh0 copy
                    yT1_ps = psp.tile([P, 2, 2 * S], F32, tag="ph", bufs=2)
                yt_src = [yT_ps[:, :, 0 : 2 * S],
                          yT1_ps if yt2_ph else yT_ps[:, :, 2 * S : ES]]
                for eh in range(2):
                    dst = yt_src[eh]
                    for e2 in range(2):
                        e = 2 * eh + e2
                        for dc in range(2):
                            for hc in range(4):
                                nc.tensor.matmul(
                                    dst[:, dc, ds(e2 * S, S)],
                                    w2_sb[:, e, hc, ds(dc * P, P)],
                                    h_slice(hc, e),
                                    start=(hc == 0),
                                    stop=(hc == 3),
                                )
                    if not yt_single:
                        # per-half copy: earlier first transpose, but the eh1
                        # matmuls carry a PSUM-bank WAR on the eh0 copy
                        yt_copy(eh)
                if yt_single:
                    nc.vector.tensor_copy(yT_sb, yT_ps)

                # transpose yT -> y [es, d] (fp16 PSUM), scale rows by the
                # dispatch normalizer on the PSUM->SBUF copy. The PSUM tile
                # is padded to a full 2KiB bank so no other tile shares the
                # bank with PE transpose writes.
                # the transpose reads yT_sb (SBUF), which the yT copy wrote
                # after the yT bank's accumulation groups closed — so reusing
                # the yT bank for the transpose output adds no ordering the
                # data deps don't already impose, and frees a PSUM bank.
                ytr_ps = psp.tile(
                    [P, 4, D], y_dt, tag="yT" if ytr_share else "ytr"
                )
                for ec in range(2):
                    for dc in range(2):
                        nc.tensor.transpose(
                            ytr_ps[:, ec, ts(dc, P)],
                            yT_sb[:, dc, ts(ec, P)],
                            ident,
                        )
                    yse = ("scalar" if ec and last_par
                           and BPC - lp_lo <= ib < lp_hi
                           and not (endgame and ib == BPC - 1) else ysc_eng)
                    scaled_copy(yse, y_sb[:, ec, :], ytr_ps[:, ec, :],
                                recip_d[:, ec : ec + 1])
                    if has_b2:
                        nc.vector.tensor_add(
                            y_sb[:, ec, :], y_sb[:, ec, :],
                            b2_tiles[g][:, ec, :],
                        )
                return (exp_lT, recip_c, y_sb)

            def stage3(ib, ctx):
                exp_lT, recip_c, y_sb = ctx
                # out [m, d] = exp_lT.T @ y, then combine normalization.
                if ou_split:
                    # separate banks per m-half: the mc1 matmuls don't carry
                    # a PSUM-bank WAR on the mc0 scale
                    ou0_ps = psp.tile([P, D], F32, tag="ou")
                    ou1_ps = psp.tile([P, D], F32, tag="ou1")
                    ou_t = [ou0_ps, ou1_ps]
                else:
                    ou_ps = psp.tile([P, 2, D], F32, tag="ou")
                    ou_t = [ou_ps[:, 0, :], ou_ps[:, 1, :]]
                out_sb = io.tile([P, 2, D], o_dt, tag="out")
                ov = out_d[ib].rearrange("p (c d) -> p c d", c=2)
                oe = last_out_eng if ib == BPC - 1 else out_eng

                def ou_mm(mc):
                    for ec in range(2):
                        nc.tensor.matmul(
                            ou_t[mc],
                            exp_lT[:, ec, ts(mc, P)],
                            y_sb[:, ec, :],
                            start=(ec == 0),
                            stop=(ec == 1),
                        )

                def ou_scale_bcast():
                    rc = recip_c.broadcast_to([P, 2, D])
                    nc.vector.tensor_tensor(
                        out_sb, ou_ps, rc, mybir.AluOpType.mult
                    )

                def ou_scale(mc):
                    if fine_last and ib >= BPC - fine_last:
                        split2([
                            (out_sb[:, mc, 0:D // 2], ou_t[mc][:, 0:D // 2]),
                            (out_sb[:, mc, D // 2 :], ou_t[mc][:, D // 2 :]),
                        ], scale=recip_c[:, mc : mc + 1])
                    else:
                        scaled_copy(oe[mc], out_sb[:, mc, :], ou_t[mc],
                                    recip_c[:, mc : mc + 1])
                    if ib == BPC - 1:
                        # last batch: SP queue is empty; ship each half as
                        # soon as its scale finishes
                        nc.sync.dma_start(out=ov[:, mc, :], in_=out_sb[:, mc, :])

                if (endgame == 3 and ib == BPC - 1) or (
                    endgame == 4 and ib >= BPC - 2
                ):
                    # endgame v3: both accumulation groups close, then ONE
                    # broadcast tensor_tensor multiply scales both halves in
                    # a single DVE op, and one full store pays a single
                    # HWDGE pass.
                    ou_mm(0)
                    ou_mm(1)
                    rc = recip_c.broadcast_to([P, 2, D])
                    nc.vector.tensor_tensor(
                        out_sb, ou_ps, rc, mybir.AluOpType.mult
                    )
                    if ib == BPC - 1:
                        nc.sync.dma_start(out=out_d[ib], in_=out_sb)
                    else:
                        eng(store_eng).dma_start(out=out_d[ib], in_=out_sb)
                    return
                if endgame and ib == BPC - 1:
                    # endgame: both accumulation groups close, the two scales
                    # run on DVE and ACT in parallel, and ONE full store pays
                    # a single HWDGE pass (two serial 625ns passes cost more
                    # than the wider transfer).
                    ou_mm(0)
                    ou_mm(1)
                    scaled_copy(last_out_eng[0], out_sb[:, 0, :], ou_t[0],
                                recip_c[:, 0:1])
                    if endgame == 2:
                        # per-half stores: mc0's HWDGE pass overlaps the
                        # mc1 scale
                        nc.sync.dma_start(out=ov[:, 0, :], in_=out_sb[:, 0, :])
                        scaled_copy(last_out_eng[1], out_sb[:, 1, :], ou_t[1],
                                    recip_c[:, 1:2])
                        nc.sync.dma_start(out=ov[:, 1, :], in_=out_sb[:, 1, :])
                    else:
                        scaled_copy(last_out_eng[1], out_sb[:, 1, :], ou_t[1],
                                    recip_c[:, 1:2])
                        nc.sync.dma_start(out=out_d[ib], in_=out_sb)
                    return
                if ou_order == "bcast":
                    ou_mm(0)
                    ou_mm(1)
                    ou_scale_bcast()
                    if ib == BPC - 1:
                        nc.sync.dma_start(out=out_d[ib], in_=out_sb)
                    else:
                        eng(store_eng).dma_start(out=out_d[ib], in_=out_sb)
                    return
                scale_after = ou_order == "scale_after" or (
                    ou_order == "last2" and ib >= BPC - 2
                )
                if not scale_after:
                    for mc in range(2):
                        ou_mm(mc)
                        ou_scale(mc)
                else:
                    # close both accumulation groups before any scale reads
                    # the bank: no WAR stall on the PE (pays off in the drain
                    # where no other batch fills the wait)
                    ou_mm(0)
                    ou_mm(1)
                    ou_scale(0)
                    ou_scale(1)
                if ib != BPC - 1:
                    # Stores ride the ACT HWDGE ring: on SP they would sit in
                    # the FIFO ahead of the next batch's weight loads and
                    # head-of-line block them.
                    eng(store_eng).dma_start(out=out_d[ib], in_=out_sb)

            # Software pipeline: stage1(ib) runs `lookahead` batches ahead of
            # stage2; stage3 trails stage2 by `s3_lag`. Each stage uses its
            # own PSUM tags so depth costs no extra PSUM.
            c1, c2 = {}, {}

            def emit_s2(j):
                c2[j] = stage2(j, c1.pop(j))

            def emit_s3(j):
                stage3(j, c2.pop(j))

            for ib in range(BPC):
                c1[ib] = stage1(ib)
                j = ib - lookahead
                if j >= 0:
                    emit_s2(j)
                if j - s3_lag >= 0:
                    emit_s3(j - s3_lag)
            for j in range(max(0, BPC - lookahead), BPC):
                emit_s2(j)
                if j - s3_lag >= 0 and j - s3_lag in c2:
                    emit_s3(j - s3_lag)
            for j in sorted(c2):
                stage3(j, c2[j])

    nc.compile()
    return nc


class _Runner:
    """Compile once per process; re-execute via a cached jitted shard_map."""

    def __init__(self, pattern, mm_dt=F32, y_dt=None, has_b2=False):
        # The Tile PSUM slot allocator is heuristic and can spuriously fail
        # near capacity; retry a few times.
        kw = {}
        if pattern == (4, 2, 1, 1):
            # hand-tuned load schedule for the common pattern; the first
            # group's load is split into expert-halves so stage2(0)'s yT
            # matmuls start off the first half ~1.5us earlier
            kw["w2_load_at"] = (2, 4, 5, 5)
            kw["w2_halves"] = 1
        last = None
        for _ in range(4):
            try:
                self.nc = build_nc(
                    pattern, mm_dt=mm_dt, y_dt=y_dt, has_b2=has_b2, **kw
                )
                break
            except ValueError as e:
                last = e
        else:
            raise last
        self.has_b2 = has_b2
        self._fn = None

    def _build_fn(self):
        import jax
        from jax.sharding import Mesh, PartitionSpec
        from jax.experimental.shard_map import shard_map
        from concourse import bass2jax
        from concourse.bass2jax import _bass_exec_p, partition_id_tensor

        bass2jax.install_neuronx_cc_hook()
        nc = self.nc
        partition_name = (
            nc.partition_id_tensor.name if nc.partition_id_tensor else None
        )
        in_names, out_names, out_avals, zero_outs = [], [], [], []
        for alloc in nc.m.functions[0].allocations:
            if not isinstance(alloc, mybir.MemoryLocationSet):
                continue
            name = alloc.memorylocations[0].name
            if alloc.kind == "ExternalInput":
                if name != partition_name:
                    in_names.append(name)
            elif alloc.kind == "ExternalOutput":
                shape = tuple(alloc.tensor_shape)
                dtype = mybir.dt.np(alloc.dtype)
                out_names.append(name)
                out_avals.append(jax.core.ShapedArray(shape, dtype))
                zero_outs.append(np.zeros(shape, dtype))
        n_params = len(in_names)
        all_in_names = list(in_names) + list(out_names)
        if partition_name is not None:
            all_in_names.append(partition_name)

        def _body(*args):
            operands = list(args)
            if partition_name is not None:
                operands.append(partition_id_tensor())
            outs = _bass_exec_p.bind(
                *operands,
                out_avals=tuple(out_avals),
                in_names=tuple(all_in_names),
                out_names=tuple(out_names),
                lowering_input_output_aliases=(),
                sim_require_finite=True,
                sim_require_nnan=True,
                nc=nc,
            )
            return tuple(outs)

        devices = jax.devices()[:N_CORES]
        assert len(devices) >= N_CORES, (
            f"need {N_CORES} NeuronCores, found {len(jax.devices())}"
        )
        mesh = Mesh(np.asarray(devices), ("core",))
        n_outs = len(out_names)
        sharded = jax.jit(
            shard_map(
                _body,
                mesh=mesh,
                in_specs=(PartitionSpec("core"),) * (n_params + n_outs),
                out_specs=(PartitionSpec("core"),) * n_outs,
                check_rep=False,
            ),
            donate_argnums=tuple(range(n_params, n_params + n_outs)),
            keep_unused=True,
        )
        self._in_names = in_names
        self._out_names = out_names
        self._out_avals = out_avals
        self._zero_outs = zero_outs
        self._fn = sharded

    def run(self, in_maps):
        """in_maps: list of N_CORES dicts name->np.ndarray. Returns per-core
        dict of outputs."""
        if self._fn is None:
            self._build_fn()
        concat_in = [
            np.concatenate([in_maps[c][nm] for c in range(N_CORES)], axis=0)
            for nm in self._in_names
        ]
        concat_zeros = [
            np.zeros((N_CORES * z.shape[0], *z.shape[1:]), z.dtype)
            for z in self._zero_outs
        ]
        out_arrs = self._fn(*concat_in, *concat_zeros)
        return [
            {
                nm: np.asarray(out_arrs[i]).reshape(
                    N_CORES, *self._out_avals[i].shape
                )[c]
                for i, nm in enumerate(self._out_names)
            }
            for c in range(N_CORES)
        ]


_runner_cache = {}


def _prep_inputs(obs, action, phi, w1, b1, w2, b2):
    obs = np.ascontiguousarray(np.asarray(obs, dtype=np.float32))
    action = np.asarray(action).astype(np.int64)
    phi = np.asarray(phi, dtype=np.float32)
    w1 = np.ascontiguousarray(np.asarray(w1, dtype=np.float32))
    b1 = np.asarray(b1, dtype=np.float32)
    w2 = np.asarray(w2, dtype=np.float32)
    b2 = np.asarray(b2, dtype=np.float32)
    if np.any(b1):
        # The device kernel folds the dispatch-softmax normalizer past the
        # ReLU, which requires b1 == 0 (true for this problem's inputs).
        # Any other input falls back to an exact host computation.
        return None
    pattern, order = solve_groups(action)
    G = len(pattern)
    obs = obs[order]
    action_s = action[order]
    # group-leader action per (core, group)
    b2g = []
    for g, s in enumerate(pattern):
        b2g += [g] * s
    grp_action = np.empty((N_CORES, G), np.int64)
    for c in range(N_CORES):
        for ib in range(BPC):
            grp_action[c, b2g[ib]] = action_s[c * BPC + ib]

    obsT = obs.transpose(0, 2, 1)
    # obs [B,M,D] -> (b, p, mc, d): m = mc*128 + p ; obsT -> (b, p, dc, m)
    obs_k = obs.reshape(B, 2, P, D).transpose(0, 2, 1, 3)
    obsT_k = obsT.reshape(B, 2, P, M).transpose(0, 2, 1, 3)
    oo_k = np.ascontiguousarray(
        np.concatenate([obs_k, obsT_k], axis=2)
    ).reshape(B, P, 4 * D)
    # phi [D,ES] -> (p, dc, es)
    phi_k = np.ascontiguousarray(
        phi.reshape(2, P, ES).transpose(1, 0, 2)
    ).reshape(P, 2 * ES)
    # w1 [E,D,H] -> (p, dc, e, h)
    w1_k = np.ascontiguousarray(
        w1.reshape(E, 2, P, H).transpose(2, 1, 0, 3)
    ).reshape(P, 2 * E * H)
    # per-group action-selected slices: w2grp [NC, G, P, E*4*D]
    w2r = w2.reshape(E, H, A, D)
    w2sel = w2r[:, :, grp_action.reshape(-1), :].transpose(2, 0, 1, 3)
    w2_k = np.ascontiguousarray(
        w2sel.reshape(N_CORES * G, E, 4, P, D).transpose(0, 3, 1, 2, 4)
    ).reshape(N_CORES, G, P, E * 4 * D)
    has_b2 = bool(np.any(b2))
    b2_k = None
    if has_b2:
        b2r = b2.reshape(E, A, D)
        b2_k = np.ascontiguousarray(
            b2r[:, grp_action.reshape(-1), :].transpose(1, 0, 2)
        ).reshape(N_CORES, G, 1, E * D)

    np_main = mybir.dt.np(MM_DT)
    np_y = mybir.dt.np(Y_DT)
    oo_k = oo_k.astype(np_main)
    phi_k = phi_k.astype(np_main)
    w1_k = w1_k.astype(np_main)
    w2_k = w2_k.astype(np_y)
    if has_b2:
        b2_k = b2_k.astype(np_y)
    in_maps = []
    for c in range(N_CORES):
        sl = slice(c * BPC, (c + 1) * BPC)
        m = {
            "oo": oo_k[sl],
            "phi": phi_k,
            "w1": w1_k,
            "w2grp": w2_k[c],
        }
        if has_b2:
            m["b2grp"] = b2_k[c]
        in_maps.append(m)
    return in_maps, has_b2, order, pattern


def get_runner(has_b2, pattern=None, mm_dt=None, y_dt=None):
    if pattern is None:
        pattern = PATTERNS[0]
    if mm_dt is None:
        mm_dt = MM_DT
    if y_dt is None:
        y_dt = Y_DT
    key = (pattern, str(mm_dt), str(y_dt), has_b2)
    if key not in _runner_cache:
        _runner_cache[key] = _Runner(
            pattern, mm_dt=mm_dt, y_dt=y_dt, has_b2=has_b2
        )
    return _runner_cache[key]


def _numpy_reference(obs, action, phi, w1, b1, w2, b2):
    obs = np.asarray(obs, np.float64)
    logits = np.einsum("bmd,des->bmes", obs, np.asarray(phi, np.float64).reshape(D, E, S))
    lmax = logits.max(axis=1, keepdims=True)
    el = np.exp(logits - lmax)
    dispatch = el / el.sum(axis=1, keepdims=True)
    lf = logits.reshape(B, M, E * S)
    ec_ = np.exp(lf - lf.max(axis=-1, keepdims=True))
    combine = (ec_ / ec_.sum(axis=-1, keepdims=True)).reshape(B, M, E, S)
    slots = np.einsum("bmd,bmes->besd", obs, dispatch)
    h = np.maximum(
        np.einsum("besd,edh->besh", slots, np.asarray(w1, np.float64))
        + np.asarray(b1, np.float64)[None, :, None, :], 0
    )
    y = np.einsum("besh,ehk->besk", h, np.asarray(w2, np.float64)) + np.asarray(
        b2, np.float64
    )[None, :, None, :]
    out = np.einsum("bmes,besk->bmk", combine, y)
    out = out.reshape(B, M, A, D).transpose(0, 2, 1, 3)
    oh = np.eye(A)[np.asarray(action).astype(np.int64)]
    return np.einsum("bamd,ba->bmd", out, oh).astype(np.float32)


def kernel(obs, action, phi, w1, b1, w2, b2):
    prep = _prep_inputs(obs, action, phi, w1, b1, w2, b2)
    if prep is None:
        return _numpy_reference(obs, action, phi, w1, b1, w2, b2)
    in_maps, has_b2, order, pattern = prep
    runner = get_runner(has_b2, pattern)
    out_k = None
    last_err = None
    for attempt in range(4):
        try:
            results = runner.run(in_maps)
        except Exception as e:  # transient device wedges recover on retry
            last_err = e
            time.sleep(2.0)
            continue
        cand = np.concatenate(
            [results[c]["out"] for c in range(N_CORES)], axis=0
        )
        out_k = cand
        # transient device glitches can yield non-finite garbage without
        # raising; for these inputs the true output is always finite, so
        # re-run rather than return it
        if np.isfinite(cand.astype(np.float32)).all():
            break
    if out_k is None:
        raise last_err
    # (b, p, mc, d) -> [B, M, D] with m = mc*128 + p; undo the action sort
    out_s = (
        out_k.astype(np.float32)
        .reshape(B, P, 2, D)
        .transpose(0, 2, 1, 3)
        .reshape(B, M, D)
    )
    out = np.empty_like(out_s)
    out[order] = out_s
    return np.ascontiguousarray(out)
